# revision 1
# baseline (speedup 1.0000x reference)
"""Trainium2 Bass kernel for LuluAttention (gated GQA attention + RoPE).

Sharding over 8 NeuronCores: core = b*4 + g where b = batch (2), g = head
group (4). Each core computes 4 Q heads + their shared KV head for one batch
element, plus the matching gate slice, and a partial o_proj output
(contraction over its 512 attn dims). Host sums the 4 partials per batch.

All on-chip tensors are kept in transposed layout ([dim, seq]) so the
attention pipeline needs no on-chip transposes:
  qT/kT [d, s]  -> scoresT[sk, sq] = kT_tile.T @ qT_chunk
  softmax over sk (partition dim): denominator via ones-matmul, broadcast of
  the reciprocal via a K=1 matmul.
  v kept straight [s, d] -> attnT[d, sq] = v_tile.T @ probsT
  agT[d, sq] = attnT * gateT * recip  feeds o_proj directly as lhsT.
RoPE rotate-half needs a cross-partition rotation by 64: done with two DMA
copies, signs folded into the host-precomputed sin table.
"""

import numpy as np
import ml_dtypes
from contextlib import ExitStack

import concourse.bass as bass
import concourse.bacc as bacc
import concourse.tile as tile
from concourse import mybir
from concourse.bass_utils import run_bass_kernel_spmd

BF16 = ml_dtypes.bfloat16

HIDDEN = 2048
B = 2
S_FULL = 2048
P = 128
CH = 512               # seq chunk width
QH = 4                 # q heads per core
DQ = QH * P            # 512 q dims per core
KT = HIDDEN // P       # 16 contraction tiles
SCALE = 1.0 / float(np.sqrt(128.0))
ROPE_THETA = 10000.0


def build_program(S=S_FULL):
    f32 = mybir.dt.float32
    bf16 = mybir.dt.bfloat16
    sig = mybir.ActivationFunctionType.Sigmoid
    expf = mybir.ActivationFunctionType.Exp

    NCH = S // CH
    ST = CH // P           # 4 seq sub-tiles per chunk

    nc = bacc.Bacc("TRN2", debug=False, target_bir_lowering=False)

    xT = nc.declare_dram_parameter("xT", [HIDDEN, S], bf16, False)
    wq = nc.declare_dram_parameter("wq", [HIDDEN, DQ], bf16, False)
    wk = nc.declare_dram_parameter("wk", [HIDDEN, P], bf16, False)
    wv = nc.declare_dram_parameter("wv", [HIDDEN, P], bf16, False)
    wg = nc.declare_dram_parameter("wg", [HIDDEN, DQ], bf16, False)
    wo = nc.declare_dram_parameter("wo", [DQ, HIDDEN], bf16, False)
    bg = nc.declare_dram_parameter("bg", [DQ], f32, False)
    cosT = nc.declare_dram_parameter("cosT", [P, S], f32, False)
    sinT = nc.declare_dram_parameter("sinT", [P, S], f32, False)
    msk = nc.declare_dram_parameter("msk", [ST, P, CH], bf16, False)
    out = nc.declare_dram_parameter("out", [S, HIDDEN], f32, True)

    with tile.TileContext(nc) as tc, ExitStack() as ctx:
        wpool = ctx.enter_context(tc.tile_pool(name="weights", bufs=1))
        xpool = ctx.enter_context(tc.tile_pool(name="xchunks", bufs=2))
        qkv = ctx.enter_context(tc.tile_pool(name="qkv", bufs=1))
        work = ctx.enter_context(tc.tile_pool(name="work", bufs=3))
        agp = ctx.enter_context(tc.tile_pool(name="agp", bufs=2))
        outp = ctx.enter_context(tc.tile_pool(name="outp", bufs=2))
        ps_mm = ctx.enter_context(tc.tile_pool(name="ps_mm", bufs=2, space="PSUM"))
        ps_sc = ctx.enter_context(tc.tile_pool(name="ps_sc", bufs=2, space="PSUM"))
        ps_at = ctx.enter_context(tc.tile_pool(name="ps_at", bufs=2, space="PSUM"))
        ps_sm = ctx.enter_context(tc.tile_pool(name="ps_sm", bufs=1, space="PSUM"))

        # ---- persistent loads ----
        wq_sb = wpool.tile([P, KT, DQ], bf16, tag="wq")
        nc.sync.dma_start(out=wq_sb, in_=wq[:, :].rearrange("(kt p) n -> p kt n", p=P))
        wk_sb = wpool.tile([P, KT, P], bf16, tag="wk")
        nc.sync.dma_start(out=wk_sb, in_=wk[:, :].rearrange("(kt p) n -> p kt n", p=P))
        wv_sb = wpool.tile([P, KT, P], bf16, tag="wv")
        nc.sync.dma_start(out=wv_sb, in_=wv[:, :].rearrange("(kt p) n -> p kt n", p=P))
        wg_sb = wpool.tile([P, KT, DQ], bf16, tag="wg")
        nc.sync.dma_start(out=wg_sb, in_=wg[:, :].rearrange("(kt p) n -> p kt n", p=P))
        wo_sb = wpool.tile([P, QH, HIDDEN], bf16, tag="wo")
        nc.sync.dma_start(out=wo_sb, in_=wo[:, :].rearrange("(dt p) n -> p dt n", p=P))
        bg_sb = wpool.tile([P, QH], f32, tag="bg")
        nc.sync.dma_start(out=bg_sb, in_=bg[:].rearrange("(h p) -> p h", p=P))
        cos_sb = wpool.tile([P, S], f32, tag="cos")
        nc.sync.dma_start(out=cos_sb, in_=cosT[:, :])
        sin_sb = wpool.tile([P, S], f32, tag="sin")
        nc.sync.dma_start(out=sin_sb, in_=sinT[:, :])
        msk_sb = wpool.tile([P, ST, CH], bf16, tag="msk")
        nc.sync.dma_start(out=msk_sb, in_=msk[:, :, :].rearrange("o p n -> p o n"))
        ones_pv = wpool.tile([P, 1], bf16, tag="ones_pv")
        nc.vector.memset(ones_pv, 1.0)
        ones_bc = wpool.tile([1, P], f32, tag="ones_bc")
        nc.vector.memset(ones_bc, 1.0)

        # persistent per-core activations (transposed layouts)
        qro = qkv.tile([P, QH, S], bf16, tag="qro")
        kro = qkv.tile([P, S], bf16, tag="kro")
        v_sb = qkv.tile([P, S // P, P], bf16, tag="v")
        gt = qkv.tile([P, QH, S], bf16, tag="gt")

        for c in range(NCH):
            cs = slice(c * CH, (c + 1) * CH)

            # ---- projections for this seq chunk ----
            xc = xpool.tile([P, KT, CH], bf16, tag="xc")
            nc.sync.dma_start(
                out=xc, in_=xT[:, cs].rearrange("(kt p) n -> p kt n", p=P)
            )

            # q heads + k, with RoPE applied out of PSUM
            for qh in range(QH + 1):
                ps = ps_mm.tile([P, CH], f32, tag="proj")
                for kt in range(KT):
                    lhs = (
                        wq_sb[:, kt, qh * P:(qh + 1) * P]
                        if qh < QH
                        else wk_sb[:, kt, :]
                    )
                    nc.tensor.matmul(
                        ps, lhs, xc[:, kt, :], start=(kt == 0), stop=(kt == KT - 1)
                    )
                qf = work.tile([P, CH], f32, tag="qf")
                nc.scalar.copy(out=qf, in_=ps)
                rot = work.tile([P, CH], f32, tag="rot")
                nc.sync.dma_start(out=rot[0:64, :], in_=qf[64:128, :])
                nc.sync.dma_start(out=rot[64:128, :], in_=qf[0:64, :])
                t1 = work.tile([P, CH], f32, tag="t1")
                nc.vector.tensor_mul(t1, qf, cos_sb[:, cs])
                t2 = work.tile([P, CH], f32, tag="t2")
                nc.vector.tensor_mul(t2, rot, sin_sb[:, cs])
                dst = qro[:, qh, cs] if qh < QH else kro[:, cs]
                nc.vector.tensor_add(dst, t1, t2)

            # gate heads: sigmoid(x @ Wg + bg), transposed layout
            for qh in range(QH):
                ps = ps_mm.tile([P, CH], f32, tag="proj")
                for kt in range(KT):
                    nc.tensor.matmul(
                        ps,
                        wg_sb[:, kt, qh * P:(qh + 1) * P],
                        xc[:, kt, :],
                        start=(kt == 0),
                        stop=(kt == KT - 1),
                    )
                nc.scalar.activation(
                    out=gt[:, qh, cs],
                    in_=ps,
                    func=sig,
                    bias=bg_sb[:, qh:qh + 1],
                    scale=1.0,
                )

            # v in straight layout [s, d]
            for st in range(ST):
                s0 = c * ST + st
                ps = ps_mm.tile([P, P], f32, tag="proj")
                for kt in range(KT):
                    nc.tensor.matmul(
                        ps,
                        xc[:, kt, st * P:(st + 1) * P],
                        wv_sb[:, kt, :],
                        start=(kt == 0),
                        stop=(kt == KT - 1),
                    )
                nc.scalar.copy(out=v_sb[:, s0, :], in_=ps)

            # ---- attention for this sq chunk ----
            ag = agp.tile([P, QH, CH], bf16, tag="ag")
            ntiles = (c + 1) * ST
            for qh in range(QH):
                at = ps_at.tile([P, CH], f32, tag="attn")
                dn = ps_sm.tile([1, CH], f32, tag="denom")
                for t in range(ntiles):
                    sc_ps = ps_sc.tile([P, CH], f32, tag="sc")
                    nc.tensor.matmul(
                        sc_ps,
                        kro[:, t * P:(t + 1) * P],
                        qro[:, qh, cs],
                        start=True,
                        stop=True,
                    )
                    pr = work.tile([P, CH], bf16, tag="probs")
                    nc.scalar.activation(out=pr, in_=sc_ps, func=expf, scale=SCALE)
                    o = t - c * ST
                    if o >= 0:
                        nc.vector.tensor_mul(pr, pr, msk_sb[:, o, :])
                    nc.tensor.matmul(
                        at, v_sb[:, t, :], pr,
                        start=(t == 0), stop=(t == ntiles - 1),
                    )
                    nc.tensor.matmul(
                        dn, ones_pv, pr,
                        start=(t == 0), stop=(t == ntiles - 1),
                    )
                rc = work.tile([1, CH], f32, tag="recip")
                nc.vector.reciprocal(rc, dn)
                bc = ps_sm.tile([P, CH], f32, tag="bcast")
                nc.tensor.matmul(bc, ones_bc, rc, start=True, stop=True)
                t3 = work.tile([P, CH], f32, tag="t3")
                nc.vector.tensor_mul(t3, at, gt[:, qh, cs])
                nc.vector.tensor_mul(ag[:, qh, :], t3, bc)

            # ---- partial o_proj for this chunk ----
            for st in range(ST):
                r0 = c * CH + st * P
                for hp in range(HIDDEN // CH // 2):
                    pss = [
                        ps_mm.tile([P, CH], f32, tag="proj", name=f"ops{hi}")
                        for hi in range(2)
                    ]
                    for dt in range(QH):
                        for hi in range(2):
                            h0 = hp * 2 + hi
                            nc.tensor.matmul(
                                pss[hi],
                                ag[:, dt, st * P:(st + 1) * P],
                                wo_sb[:, dt, h0 * CH:(h0 + 1) * CH],
                                start=(dt == 0),
                                stop=(dt == QH - 1),
                            )
                    for hi in range(2):
                        h0 = hp * 2 + hi
                        ob = outp.tile([P, CH], f32, tag="ob")
                        nc.vector.tensor_copy(out=ob, in_=pss[hi])
                        nc.sync.dma_start(
                            out=out[r0:r0 + P, h0 * CH:(h0 + 1) * CH], in_=ob
                        )

    nc.finalize()
    return nc


_PROGRAMS = {}


def _get_program(S=S_FULL):
    if S not in _PROGRAMS:
        _PROGRAMS[S] = build_program(S)
    return _PROGRAMS[S]


def _host_tables(position_ids_b, S):
    pos = np.asarray(position_ids_b, dtype=np.float32)  # [S]
    inv = 1.0 / (ROPE_THETA ** (np.arange(0, P, 2, dtype=np.float32) / P))  # [64]
    ang = np.concatenate([inv, inv]).astype(np.float32)[:, None] * pos[None, :]
    cosT = np.cos(ang).astype(np.float32)
    sgn = np.where(np.arange(P) < 64, -1.0, 1.0).astype(np.float32)
    sinT = (np.sin(ang) * sgn[:, None]).astype(np.float32)
    return cosT, sinT


def _causal_masks():
    o = np.arange(CH // P)[:, None, None]
    r = np.arange(P)[None, :, None]
    j = np.arange(CH)[None, None, :]
    return ((P * o + r) <= j).astype(BF16)


def make_in_maps(x, position_ids, Wq, Wk, Wv, Wo, Wg, bg, S=S_FULL):
    x = np.asarray(x, dtype=np.float32)
    msk = _causal_masks()
    maps = []
    xT_b = [np.ascontiguousarray(x[b, :S].T).astype(BF16) for b in range(B)]
    tabs = [_host_tables(np.asarray(position_ids)[b, :S], S) for b in range(B)]
    Wq = np.asarray(Wq, np.float32)
    Wk = np.asarray(Wk, np.float32)
    Wv = np.asarray(Wv, np.float32)
    Wo = np.asarray(Wo, np.float32)
    Wg = np.asarray(Wg, np.float32)
    bg = np.asarray(bg, np.float32)
    for core in range(8):
        b, g = core // 4, core % 4
        cosT, sinT = tabs[b]
        maps.append({
            "xT": xT_b[b],
            "wq": np.ascontiguousarray(Wq[:, g * DQ:(g + 1) * DQ]).astype(BF16),
            "wk": np.ascontiguousarray(Wk[:, g * P:(g + 1) * P]).astype(BF16),
            "wv": np.ascontiguousarray(Wv[:, g * P:(g + 1) * P]).astype(BF16),
            "wg": np.ascontiguousarray(Wg[:, g * DQ:(g + 1) * DQ]).astype(BF16),
            "wo": np.ascontiguousarray(Wo[g * DQ:(g + 1) * DQ, :]).astype(BF16),
            "bg": np.ascontiguousarray(bg[g * DQ:(g + 1) * DQ]),
            "cosT": cosT,
            "sinT": sinT,
            "msk": msk,
        })
    return maps


def run(inputs, S=S_FULL, trace=False, **kw):
    nc = _get_program(S)
    maps = make_in_maps(S=S, **inputs)
    res = run_bass_kernel_spmd(nc, maps, core_ids=list(range(8)), trace=trace, **kw)
    out = np.zeros((B, S, HIDDEN), np.float32)
    for core in range(8):
        out[core // 4] += np.asarray(res.results[core]["out"], np.float32)
    return out, res


def kernel(x, position_ids, Wq, Wk, Wv, Wo, Wg, bg):
    out, _ = run(dict(x=x, position_ids=position_ids, Wq=Wq, Wk=Wk, Wv=Wv,
                      Wo=Wo, Wg=Wg, bg=bg))
    return out



# revision 6
# speedup vs baseline: 1.4781x; 1.4781x over previous
"""Trainium2 Bass kernel for LuluAttention v2 (fp8 probs/V DoubleRow + engine rebalance).

Sharding (unchanged from baseline): core = b*4 + g; each core computes 4 Q heads
+ their shared KV head for one batch element, plus the matching gate slice, and
a partial o_proj (contraction over its 512 attn dims). Host sums 4 partials.

Changes vs 553us baseline:
  * probs stored fp8 e4m3 in [sk, 2, sq] pair layout; V stored fp8 PLUS an fp8
    residual correction (v = v8 + vr8, error ~0.1%). The P@V and denominator
    matmuls run fp8 DoubleRow: 3x256cy per tile pair instead of bf16 3x512cy.
    Only the probs quantization (~1.5e-2 max rel on full output) is exposed.
  * everything else stays bf16: full-fp8 was measured at 5.2e-2 rel err (>2e-2
    tolerance) since every fp8 cast feeds ~2-3% straight into the output.
  * broadcast-of-reciprocal matmul in float32r: 1 cy/row instead of fp32's 4.
  * reciprocal via reciprocal_approx_fast straight off PSUM (old: 3.3us
    single-lane InstReciprocal + fp32 broadcast = ~85us of waste).
  * RoPE sin-multiply moved to the idle GpSimd(Pool) engine.
  * o_proj of chunk c deferred past the projections of chunk c+1 so the PE
    isn't stalled on the DVE gating epilogue at chunk boundaries.
  * o_proj PSUM->SBUF copies alternate DVE/ACT; output partials in bf16
    (halves output DMA), host sums in f32.
"""

import numpy as np
import ml_dtypes
from contextlib import ExitStack

import concourse.bass as bass
import concourse.bacc as bacc
import concourse.tile as tile
from concourse import mybir
from concourse.bass_utils import run_bass_kernel_spmd

BF16 = ml_dtypes.bfloat16
FP8 = ml_dtypes.float8_e4m3  # TRN float8e4 (max normal 240)

HIDDEN = 2048
B = 2
S_FULL = 2048
P = 128
CH = 512               # seq chunk width
QH = 4                 # q heads per core
DQ = QH * P            # 512 q dims per core
KT = HIDDEN // P       # 16 contraction tiles
SCALE = 1.0 / float(np.sqrt(128.0))
ROPE_THETA = 10000.0
VS = 16.0              # v scale: v8 fp8 holds 16*v, recip folds 1/16

DR = mybir.MatmulPerfMode.DoubleRow


def build_program(S=S_FULL):
    f32 = mybir.dt.float32
    f32r = mybir.dt.float32r
    bf16 = mybir.dt.bfloat16
    fp8 = mybir.dt.float8e4
    sig = mybir.ActivationFunctionType.Sigmoid
    expf = mybir.ActivationFunctionType.Exp

    NCH = S // CH
    ST = CH // P           # 4 seq sub-tiles per chunk

    nc = bacc.Bacc("TRN2", debug=False, target_bir_lowering=False)

    xT = nc.declare_dram_parameter("xT", [HIDDEN, S], bf16, False)
    wq = nc.declare_dram_parameter("wq", [HIDDEN, DQ], bf16, False)
    wk = nc.declare_dram_parameter("wk", [HIDDEN, P], bf16, False)
    wv = nc.declare_dram_parameter("wv", [HIDDEN, P], bf16, False)
    wg = nc.declare_dram_parameter("wg", [HIDDEN, DQ], bf16, False)
    wo = nc.declare_dram_parameter("wo", [DQ, HIDDEN], bf16, False)
    bg = nc.declare_dram_parameter("bg", [DQ], f32, False)
    cosT = nc.declare_dram_parameter("cosT", [P, S], f32, False)
    sinT = nc.declare_dram_parameter("sinT", [P, S], f32, False)
    msk = nc.declare_dram_parameter("msk", [ST, P, CH], bf16, False)
    out = nc.declare_dram_parameter("out", [S, HIDDEN], bf16, True)

    with tile.TileContext(nc) as tc, ExitStack() as ctx:
        wpool = ctx.enter_context(tc.tile_pool(name="weights", bufs=1))
        xpool = ctx.enter_context(tc.tile_pool(name="xchunks", bufs=2))
        qkv = ctx.enter_context(tc.tile_pool(name="qkv", bufs=1))
        work = ctx.enter_context(tc.tile_pool(name="work", bufs=3))
        prp = ctx.enter_context(tc.tile_pool(name="probs", bufs=3))
        agp = ctx.enter_context(tc.tile_pool(name="agp", bufs=2))
        outp = ctx.enter_context(tc.tile_pool(name="outp", bufs=3))
        ps_mm = ctx.enter_context(tc.tile_pool(name="ps_mm", bufs=2, space="PSUM"))
        ps_sc = ctx.enter_context(tc.tile_pool(name="ps_sc", bufs=2, space="PSUM"))
        ps_at = ctx.enter_context(tc.tile_pool(name="ps_at", bufs=2, space="PSUM"))
        ps_sm = ctx.enter_context(tc.tile_pool(name="ps_sm", bufs=1, space="PSUM"))

        # ---- persistent loads ----
        wq_sb = wpool.tile([P, KT, DQ], bf16, tag="wq")
        nc.sync.dma_start(out=wq_sb, in_=wq[:, :].rearrange("(kt p) n -> p kt n", p=P))
        wk_sb = wpool.tile([P, KT, P], bf16, tag="wk")
        nc.sync.dma_start(out=wk_sb, in_=wk[:, :].rearrange("(kt p) n -> p kt n", p=P))
        wv_sb = wpool.tile([P, KT, P], bf16, tag="wv")
        nc.sync.dma_start(out=wv_sb, in_=wv[:, :].rearrange("(kt p) n -> p kt n", p=P))
        wg_sb = wpool.tile([P, KT, DQ], bf16, tag="wg")
        nc.sync.dma_start(out=wg_sb, in_=wg[:, :].rearrange("(kt p) n -> p kt n", p=P))
        wo_sb = wpool.tile([P, QH, HIDDEN], bf16, tag="wo")
        nc.sync.dma_start(out=wo_sb, in_=wo[:, :].rearrange("(dt p) n -> p dt n", p=P))
        bg_sb = wpool.tile([P, QH], f32, tag="bg")
        nc.sync.dma_start(out=bg_sb, in_=bg[:].rearrange("(h p) -> p h", p=P))
        cos_sb = wpool.tile([P, S], f32, tag="cos")
        nc.sync.dma_start(out=cos_sb, in_=cosT[:, :])
        sin_sb = wpool.tile([P, S], f32, tag="sin")
        nc.sync.dma_start(out=sin_sb, in_=sinT[:, :])
        msk_sb = wpool.tile([P, ST, CH], bf16, tag="msk")
        nc.sync.dma_start(out=msk_sb, in_=msk[:, :, :].rearrange("o p n -> p o n"))
        # DoubleRow ldweights needs the pair-dim step 16B-aligned, so the
        # ones column lives in a [P, 2, 16] tile sliced to [:, :, 0:1]
        ones_pv_t = wpool.tile([P, 2, 16], fp8, tag="ones_pv")
        nc.vector.memset(ones_pv_t, 1.0)
        ones_pv = ones_pv_t[:, :, 0:1]
        ones_bc = wpool.tile([1, P], bf16, tag="ones_bc")
        nc.vector.memset(ones_bc, 1.0 / VS)
        expbias = wpool.tile([P, 1], f32, tag="expbias")
        nc.vector.memset(expbias, -2.0)

        # persistent per-core activations (transposed layouts)
        qro = qkv.tile([P, QH, S], bf16, tag="qro")
        kro = qkv.tile([P, S], bf16, tag="kro")
        v8 = qkv.tile([P, S // P, P], fp8, tag="v8")
        vr8 = qkv.tile([P, S // P, P], fp8, tag="vr8")
        gt = qkv.tile([P, QH, S], bf16, tag="gt")

        def o_proj(c, ag):
            # partial o_proj for chunk c; PSUM->SBUF copies alternate DVE/ACT
            for st in range(ST):
                r0 = c * CH + st * P
                for hb in range(HIDDEN // CH):
                    ops = ps_mm.tile([P, CH], f32, tag="proj")
                    for dt in range(QH):
                        nc.tensor.matmul(
                            ops,
                            ag[:, dt, st * P:(st + 1) * P],
                            wo_sb[:, dt, hb * CH:(hb + 1) * CH],
                            start=(dt == 0),
                            stop=(dt == QH - 1),
                        )
                    ob = outp.tile([P, CH], bf16, tag="ob")
                    if hb % 2 == 0:
                        nc.vector.tensor_copy(out=ob, in_=ops)
                    else:
                        nc.scalar.copy(out=ob, in_=ops)
                    nc.sync.dma_start(
                        out=out[r0:r0 + P, hb * CH:(hb + 1) * CH], in_=ob
                    )

        prev = None  # (c, ag) pending o_proj
        for c in range(NCH):
            cs = slice(c * CH, (c + 1) * CH)

            # ---- projections for this seq chunk ----
            xc = xpool.tile([P, KT, CH], bf16, tag="xc")
            nc.sync.dma_start(
                out=xc, in_=xT[:, cs].rearrange("(kt p) n -> p kt n", p=P)
            )

            # q heads + k, with RoPE applied out of PSUM
            for qh in range(QH + 1):
                ps = ps_mm.tile([P, CH], f32, tag="proj")
                for kt in range(KT):
                    lhs = (
                        wq_sb[:, kt, qh * P:(qh + 1) * P]
                        if qh < QH
                        else wk_sb[:, kt, :]
                    )
                    nc.tensor.matmul(
                        ps, lhs, xc[:, kt, :], start=(kt == 0), stop=(kt == KT - 1)
                    )
                qf = work.tile([P, CH], f32, tag="qf")
                nc.scalar.copy(out=qf, in_=ps)
                rot = work.tile([P, CH], f32, tag="rot")
                nc.sync.dma_start(out=rot[0:64, :], in_=qf[64:128, :])
                nc.sync.dma_start(out=rot[64:128, :], in_=qf[0:64, :])
                t1 = work.tile([P, CH], f32, tag="t1")
                nc.vector.tensor_mul(t1, qf, cos_sb[:, cs])
                t2 = work.tile([P, CH], f32, tag="t2")
                nc.gpsimd.tensor_mul(t2, rot, sin_sb[:, cs])
                dst = qro[:, qh, cs] if qh < QH else kro[:, cs]
                nc.vector.tensor_add(dst, t1, t2)

            # gate heads: sigmoid(x @ Wg + bg), transposed layout
            for qh in range(QH):
                ps = ps_mm.tile([P, CH], f32, tag="proj")
                for kt in range(KT):
                    nc.tensor.matmul(
                        ps,
                        wg_sb[:, kt, qh * P:(qh + 1) * P],
                        xc[:, kt, :],
                        start=(kt == 0),
                        stop=(kt == KT - 1),
                    )
                nc.scalar.activation(
                    out=gt[:, qh, cs],
                    in_=ps,
                    func=sig,
                    bias=bg_sb[:, qh:qh + 1],
                    scale=1.0,
                )

            # v in straight layout [s, d]: psum holds 16*v; store fp8 + residual
            for st in range(ST):
                s0 = c * ST + st
                ps = ps_mm.tile([P, P], f32, tag="proj")
                for kt in range(KT):
                    nc.tensor.matmul(
                        ps,
                        xc[:, kt, st * P:(st + 1) * P],
                        wv_sb[:, kt, :],
                        start=(kt == 0),
                        stop=(kt == KT - 1),
                    )
                nc.vector.tensor_copy(out=v8[:, s0, :], in_=ps)
                nc.vector.scalar_tensor_tensor(
                    out=vr8[:, s0, :],
                    in0=v8[:, s0, :],
                    scalar=-1.0,
                    in1=ps,
                    op0=mybir.AluOpType.mult,
                    op1=mybir.AluOpType.add,
                )

            # deferred o_proj of the previous chunk: keeps PE busy while this
            # chunk's attention epilogue (DVE gating) finishes
            if prev is not None:
                o_proj(*prev)

            # ---- attention for this sq chunk ----
            ag = agp.tile([P, QH, CH], bf16, tag="ag")
            npairs = (c + 1) * ST // 2
            for qh in range(QH):
                at = ps_at.tile([P, CH], f32, tag="attn")
                dn = ps_sm.tile([1, CH], f32, tag="denom")
                for pair in range(npairs):
                    pr2 = prp.tile([P, 2, CH], fp8, tag="pr")
                    for i in range(2):
                        t = 2 * pair + i
                        sc_ps = ps_sc.tile([P, CH], f32, tag="sc")
                        nc.tensor.matmul(
                            sc_ps,
                            kro[:, t * P:(t + 1) * P],
                            qro[:, qh, cs],
                            start=True,
                            stop=True,
                        )
                        # bias -2: softmax-invariant shift keeping exp() safely
                        # under fp8 e4m3 max normal 240 (no saturation on TRN:
                        # overflow -> Inf -> NaN through the normalizer)
                        nc.scalar.activation(
                            out=pr2[:, i, :], in_=sc_ps, func=expf, scale=SCALE,
                            bias=expbias[:, 0:1],
                        )
                        o = t - c * ST
                        if o >= 0:
                            nc.vector.tensor_mul(
                                pr2[:, i, :], pr2[:, i, :], msk_sb[:, o, :]
                            )
                    v2s = slice(2 * pair, 2 * pair + 2)
                    nc.tensor.matmul(
                        at, v8[:, v2s, :], pr2,
                        start=(pair == 0), stop=False, perf_mode=DR,
                    )
                    nc.tensor.matmul(
                        at, vr8[:, v2s, :], pr2,
                        start=False, stop=(pair == npairs - 1), perf_mode=DR,
                    )
                    nc.tensor.matmul(
                        dn, ones_pv, pr2,
                        start=(pair == 0), stop=(pair == npairs - 1), perf_mode=DR,
                    )
                rc = work.tile([1, CH], f32, tag="rc")
                nc.vector.reciprocal_approx_fast(out=rc, in_=dn)
                rcb = work.tile([1, CH], bf16, tag="rcb")
                nc.vector.tensor_copy(out=rcb, in_=rc)
                bc = ps_sm.tile([P, CH], f32, tag="bcast")
                nc.tensor.matmul(bc, ones_bc, rcb, start=True, stop=True)
                t3 = work.tile([P, CH], f32, tag="t3")
                nc.vector.tensor_mul(t3, at, gt[:, qh, cs])
                nc.vector.tensor_mul(ag[:, qh, :], t3, bc)

            prev = (c, ag)

        o_proj(*prev)

    nc.finalize()
    return nc


_PROGRAMS = {}


def _get_program(S=S_FULL):
    if S not in _PROGRAMS:
        _PROGRAMS[S] = build_program(S)
    return _PROGRAMS[S]


def _host_tables(position_ids_b, S):
    pos = np.asarray(position_ids_b, dtype=np.float32)  # [S]
    inv = 1.0 / (ROPE_THETA ** (np.arange(0, P, 2, dtype=np.float32) / P))  # [64]
    ang = np.concatenate([inv, inv]).astype(np.float32)[:, None] * pos[None, :]
    cosT = np.cos(ang).astype(np.float32)
    sgn = np.where(np.arange(P) < 64, -1.0, 1.0).astype(np.float32)
    sinT = (np.sin(ang) * sgn[:, None]).astype(np.float32)
    return cosT, sinT


def _causal_masks():
    o = np.arange(CH // P)[:, None, None]
    r = np.arange(P)[None, :, None]
    j = np.arange(CH)[None, None, :]
    return ((P * o + r) <= j).astype(BF16)


def make_in_maps(x, position_ids, Wq, Wk, Wv, Wo, Wg, bg, S=S_FULL):
    x = np.asarray(x, dtype=np.float32)
    msk = _causal_masks()
    maps = []
    xT_b = [np.ascontiguousarray(x[b, :S].T).astype(BF16) for b in range(B)]
    tabs = [_host_tables(np.asarray(position_ids)[b, :S], S) for b in range(B)]
    Wq = np.asarray(Wq, np.float32)
    Wk = np.asarray(Wk, np.float32)
    Wv = np.asarray(Wv, np.float32)
    Wo = np.asarray(Wo, np.float32)
    Wg = np.asarray(Wg, np.float32)
    bg = np.asarray(bg, np.float32)
    for core in range(8):
        b, g = core // 4, core % 4
        cosT, sinT = tabs[b]
        maps.append({
            "xT": xT_b[b],
            "wq": np.ascontiguousarray(Wq[:, g * DQ:(g + 1) * DQ]).astype(BF16),
            "wk": np.ascontiguousarray(Wk[:, g * P:(g + 1) * P]).astype(BF16),
            "wv": np.ascontiguousarray(VS * Wv[:, g * P:(g + 1) * P]).astype(BF16),
            "wg": np.ascontiguousarray(Wg[:, g * DQ:(g + 1) * DQ]).astype(BF16),
            "wo": np.ascontiguousarray(Wo[g * DQ:(g + 1) * DQ, :]).astype(BF16),
            "bg": np.ascontiguousarray(bg[g * DQ:(g + 1) * DQ]),
            "cosT": cosT,
            "sinT": sinT,
            "msk": msk,
        })
    return maps


def run(inputs, S=S_FULL, trace=False, **kw):
    nc = _get_program(S)
    maps = make_in_maps(S=S, **inputs)
    res = run_bass_kernel_spmd(nc, maps, core_ids=list(range(8)), trace=trace, **kw)
    out = np.zeros((B, S, HIDDEN), np.float32)
    for core in range(8):
        out[core // 4] += np.asarray(res.results[core]["out"], np.float32)
    return out, res


def kernel(x, position_ids, Wq, Wk, Wv, Wo, Wg, bg):
    out, _ = run(dict(x=x, position_ids=position_ids, Wq=Wq, Wk=Wk, Wv=Wv,
                      Wo=Wo, Wg=Wg, bg=bg))
    return out


# revision 7
# speedup vs baseline: 1.5635x; 1.0578x over previous
"""Trainium2 Bass kernel for LuluAttention v3 (v2 + software pipelining).

v3 changes vs v2 (374 us):
  * scores for pair p+1 issue before the PV/denominator matmuls of pair p, so
    the Act-engine exp latency hides behind score streaming (ps_sc bufs=3;
    the reciprocal broadcast shares the ps_sc ring).
  * projections of chunk c+1 and the o_proj of chunk c-1 are interleaved into
    attention(c)'s instruction stream (one piece per qh block), filling the
    PE while ACT chews exp and filling ACT (sigmoid/copies) while PE projects.
  * causal column skip: diagonal score tiles only compute/exp columns >= o*128
    (the mask multiply still covers the full width, zeroing stale fp8 data).
  * per-chunk tiles for qro/gt (pools) and kro/v8/vr8 (per-chunk tags) so the
    interleaving can't create false whole-tile write-after-read dependencies.

Numerics identical to v2: bf16 everywhere except fp8 probs (exp shifted -2)
x (fp8 v + fp8 residual) for the PV and denominator DoubleRow matmuls.
"""

import numpy as np
import ml_dtypes
from collections import deque
from contextlib import ExitStack

import concourse.bass as bass
import concourse.bacc as bacc
import concourse.tile as tile
from concourse import mybir
from concourse.bass_utils import run_bass_kernel_spmd

BF16 = ml_dtypes.bfloat16
FP8 = ml_dtypes.float8_e4m3

HIDDEN = 2048
B = 2
S_FULL = 2048
P = 128
CH = 512
QH = 4
DQ = QH * P
KT = HIDDEN // P
SCALE = 1.0 / float(np.sqrt(128.0))
ROPE_THETA = 10000.0
VS = 16.0

DR = mybir.MatmulPerfMode.DoubleRow


def build_program(S=S_FULL):
    f32 = mybir.dt.float32
    bf16 = mybir.dt.bfloat16
    fp8 = mybir.dt.float8e4
    sig = mybir.ActivationFunctionType.Sigmoid
    expf = mybir.ActivationFunctionType.Exp

    NCH = S // CH
    ST = CH // P

    nc = bacc.Bacc("TRN2", debug=False, target_bir_lowering=False)

    xT = nc.declare_dram_parameter("xT", [HIDDEN, S], bf16, False)
    wq = nc.declare_dram_parameter("wq", [HIDDEN, DQ], bf16, False)
    wk = nc.declare_dram_parameter("wk", [HIDDEN, P], bf16, False)
    wv = nc.declare_dram_parameter("wv", [HIDDEN, P], bf16, False)
    wg = nc.declare_dram_parameter("wg", [HIDDEN, DQ], bf16, False)
    wo = nc.declare_dram_parameter("wo", [DQ, HIDDEN], bf16, False)
    bg = nc.declare_dram_parameter("bg", [DQ], f32, False)
    cosT = nc.declare_dram_parameter("cosT", [P, S], f32, False)
    sinT = nc.declare_dram_parameter("sinT", [P, S], f32, False)
    msk = nc.declare_dram_parameter("msk", [ST, P, CH], bf16, False)
    out = nc.declare_dram_parameter("out", [S, HIDDEN], bf16, True)

    with tile.TileContext(nc) as tc, ExitStack() as ctx:
        wpool = ctx.enter_context(tc.tile_pool(name="weights", bufs=1))
        xpool = ctx.enter_context(tc.tile_pool(name="xchunks", bufs=2))
        qkv = ctx.enter_context(tc.tile_pool(name="qkv", bufs=1))
        qrop = ctx.enter_context(tc.tile_pool(name="qrop", bufs=2))
        gtp = ctx.enter_context(tc.tile_pool(name="gtp", bufs=2))
        work = ctx.enter_context(tc.tile_pool(name="work", bufs=3))
        prp = ctx.enter_context(tc.tile_pool(name="probs", bufs=3))
        agp = ctx.enter_context(tc.tile_pool(name="agp", bufs=2))
        outp = ctx.enter_context(tc.tile_pool(name="outp", bufs=3))
        ps_mm = ctx.enter_context(tc.tile_pool(name="ps_mm", bufs=2, space="PSUM"))
        ps_sc = ctx.enter_context(tc.tile_pool(name="ps_sc", bufs=3, space="PSUM"))
        ps_at = ctx.enter_context(tc.tile_pool(name="ps_at", bufs=2, space="PSUM"))
        ps_dn = ctx.enter_context(tc.tile_pool(name="ps_dn", bufs=1, space="PSUM"))

        # ---- persistent loads ----
        wq_sb = wpool.tile([P, KT, DQ], bf16, tag="wq")
        nc.sync.dma_start(out=wq_sb, in_=wq[:, :].rearrange("(kt p) n -> p kt n", p=P))
        wk_sb = wpool.tile([P, KT, P], bf16, tag="wk")
        nc.sync.dma_start(out=wk_sb, in_=wk[:, :].rearrange("(kt p) n -> p kt n", p=P))
        wv_sb = wpool.tile([P, KT, P], bf16, tag="wv")
        nc.sync.dma_start(out=wv_sb, in_=wv[:, :].rearrange("(kt p) n -> p kt n", p=P))
        wg_sb = wpool.tile([P, KT, DQ], bf16, tag="wg")
        nc.sync.dma_start(out=wg_sb, in_=wg[:, :].rearrange("(kt p) n -> p kt n", p=P))
        wo_sb = wpool.tile([P, QH, HIDDEN], bf16, tag="wo")
        nc.sync.dma_start(out=wo_sb, in_=wo[:, :].rearrange("(dt p) n -> p dt n", p=P))
        bg_sb = wpool.tile([P, QH], f32, tag="bg")
        nc.sync.dma_start(out=bg_sb, in_=bg[:].rearrange("(h p) -> p h", p=P))
        cos_sb = wpool.tile([P, S], f32, tag="cos")
        nc.sync.dma_start(out=cos_sb, in_=cosT[:, :])
        sin_sb = wpool.tile([P, S], f32, tag="sin")
        nc.sync.dma_start(out=sin_sb, in_=sinT[:, :])
        msk_sb = wpool.tile([P, ST, CH], bf16, tag="msk")
        nc.sync.dma_start(out=msk_sb, in_=msk[:, :, :].rearrange("o p n -> p o n"))
        ones_pv_t = wpool.tile([P, 2, 16], fp8, tag="ones_pv")
        nc.vector.memset(ones_pv_t, 1.0)
        ones_pv = ones_pv_t[:, :, 0:1]
        ones_bc = wpool.tile([1, P], bf16, tag="ones_bc")
        nc.vector.memset(ones_bc, 1.0 / VS)
        expbias = wpool.tile([P, 1], f32, tag="expbias")
        nc.vector.memset(expbias, -2.0)

        # per-chunk persistent K/V (separate tags avoid cross-chunk WAR deps)
        kro_t = [
            qkv.tile([P, CH], bf16, tag=f"kro{c}", name=f"kro{c}")
            for c in range(NCH)
        ]
        v8_t = [
            qkv.tile([P, ST, P], fp8, tag=f"v8{c}", name=f"v8{c}")
            for c in range(NCH)
        ]
        vr8_t = [
            qkv.tile([P, ST, P], fp8, tag=f"vr8{c}", name=f"vr8{c}")
            for c in range(NCH)
        ]

        xc_t = {}

        def load_xc(c):
            xc = xpool.tile([P, KT, CH], bf16, tag="xc")
            nc.sync.dma_start(
                out=xc,
                in_=xT[:, c * CH:(c + 1) * CH].rearrange("(kt p) n -> p kt n", p=P),
            )
            xc_t[c] = xc

        def qk_piece(c, qh, qro_c):
            def run():
                cs = slice(c * CH, (c + 1) * CH)
                ps = ps_mm.tile([P, CH], f32, tag="proj")
                for kt in range(KT):
                    lhs = (
                        wq_sb[:, kt, qh * P:(qh + 1) * P]
                        if qh < QH
                        else wk_sb[:, kt, :]
                    )
                    nc.tensor.matmul(
                        ps, lhs, xc_t[c][:, kt, :],
                        start=(kt == 0), stop=(kt == KT - 1),
                    )
                qf = work.tile([P, CH], f32, tag="qf")
                nc.scalar.copy(out=qf, in_=ps)
                rot = work.tile([P, CH], f32, tag="rot")
                nc.sync.dma_start(out=rot[0:64, :], in_=qf[64:128, :])
                nc.sync.dma_start(out=rot[64:128, :], in_=qf[0:64, :])
                t1 = work.tile([P, CH], f32, tag="t1")
                nc.vector.tensor_mul(t1, qf, cos_sb[:, cs])
                t2 = work.tile([P, CH], f32, tag="t2")
                nc.gpsimd.tensor_mul(t2, rot, sin_sb[:, cs])
                dst = qro_c[:, qh, :] if qh < QH else kro_t[c][:, :]
                nc.vector.tensor_add(dst, t1, t2)
            return run

        def gate_piece(c, qh, gt_c):
            def run():
                ps = ps_mm.tile([P, CH], f32, tag="proj")
                for kt in range(KT):
                    nc.tensor.matmul(
                        ps,
                        wg_sb[:, kt, qh * P:(qh + 1) * P],
                        xc_t[c][:, kt, :],
                        start=(kt == 0),
                        stop=(kt == KT - 1),
                    )
                nc.scalar.activation(
                    out=gt_c[:, qh, :], in_=ps, func=sig,
                    bias=bg_sb[:, qh:qh + 1], scale=1.0,
                )
            return run

        def v_piece(c, st):
            def run():
                ps = ps_mm.tile([P, P], f32, tag="proj")
                for kt in range(KT):
                    nc.tensor.matmul(
                        ps,
                        xc_t[c][:, kt, st * P:(st + 1) * P],
                        wv_sb[:, kt, :],
                        start=(kt == 0),
                        stop=(kt == KT - 1),
                    )
                nc.vector.tensor_copy(out=v8_t[c][:, st, :], in_=ps)
                nc.vector.scalar_tensor_tensor(
                    out=vr8_t[c][:, st, :],
                    in0=v8_t[c][:, st, :],
                    scalar=-1.0,
                    in1=ps,
                    op0=mybir.AluOpType.mult,
                    op1=mybir.AluOpType.add,
                )
            return run

        def proj_pieces(c, qro_c, gt_c):
            return (
                [qk_piece(c, qh, qro_c) for qh in range(QH + 1)]
                + [gate_piece(c, qh, gt_c) for qh in range(QH)]
                + [v_piece(c, st) for st in range(ST)]
            )

        def oproj_pieces(c, ag):
            def piece(st, hb):
                def run():
                    r0 = c * CH + st * P
                    ops = ps_mm.tile([P, CH], f32, tag="proj")
                    for dt in range(QH):
                        nc.tensor.matmul(
                            ops,
                            ag[:, dt, st * P:(st + 1) * P],
                            wo_sb[:, dt, hb * CH:(hb + 1) * CH],
                            start=(dt == 0),
                            stop=(dt == QH - 1),
                        )
                    ob = outp.tile([P, CH], bf16, tag="ob")
                    if hb % 2 == 0:
                        nc.vector.tensor_copy(out=ob, in_=ops)
                    else:
                        nc.scalar.copy(out=ob, in_=ops)
                    nc.sync.dma_start(
                        out=out[r0:r0 + P, hb * CH:(hb + 1) * CH], in_=ob
                    )
                return run
            return [piece(st, hb) for st in range(ST) for hb in range(HIDDEN // CH)]

        def attention(c, qro_c, gt_c, ag, filler):
            npairs = (c + 1) * ST // 2
            popped = 0
            ntot = len(filler)
            for qh in range(QH):
                def scores(pair):
                    pr2 = prp.tile([P, 2, CH], fp8, tag="pr")
                    for i in range(2):
                        t = 2 * pair + i
                        o = t - c * ST
                        lo = o * P if o > 0 else 0
                        sc_ps = ps_sc.tile([P, CH], f32, tag="sc")
                        nc.tensor.matmul(
                            sc_ps[:, lo:],
                            kro_t[t // ST][:, (t % ST) * P:(t % ST + 1) * P],
                            qro_c[:, qh, lo:],
                            start=True,
                            stop=True,
                        )
                        nc.scalar.activation(
                            out=pr2[:, i, lo:], in_=sc_ps[:, lo:], func=expf,
                            scale=SCALE, bias=expbias[:, 0:1],
                        )
                        if o >= 0:
                            # causal boundary crosses only cols [128o, 128o+128);
                            # left of that is fully masked (zeroed on gpsimd),
                            # right is fully unmasked
                            if lo > 0:
                                nc.gpsimd.memset(pr2[:, i, 0:lo], 0.0)
                            nc.vector.tensor_mul(
                                pr2[:, i, lo:lo + P],
                                pr2[:, i, lo:lo + P],
                                msk_sb[:, o, lo:lo + P],
                            )
                    return pr2

                def pv(pair, pr2, at, dn):
                    tc0 = 2 * pair
                    cc = tc0 // ST
                    vsl = slice(tc0 % ST, tc0 % ST + 2)
                    nc.tensor.matmul(
                        at, v8_t[cc][:, vsl, :], pr2,
                        start=(pair == 0), stop=False, perf_mode=DR,
                    )
                    nc.tensor.matmul(
                        at, vr8_t[cc][:, vsl, :], pr2,
                        start=False, stop=(pair == npairs - 1), perf_mode=DR,
                    )
                    nc.tensor.matmul(
                        dn, ones_pv, pr2,
                        start=(pair == 0), stop=(pair == npairs - 1), perf_mode=DR,
                    )

                at = ps_at.tile([P, CH], f32, tag="attn")
                dn = ps_dn.tile([1, CH], f32, tag="denom")
                pr_prev = scores(0)
                for pair in range(1, npairs):
                    pr_cur = scores(pair)
                    pv(pair - 1, pr_prev, at, dn)
                    pr_prev = pr_cur
                pv(npairs - 1, pr_prev, at, dn)

                rc = work.tile([1, CH], f32, tag="rc")
                nc.vector.reciprocal_approx_fast(out=rc, in_=dn)
                rcb = work.tile([1, CH], bf16, tag="rcb")
                nc.vector.tensor_copy(out=rcb, in_=rc)
                bc = ps_sc.tile([P, CH], f32, tag="sc")
                nc.tensor.matmul(bc, ones_bc, rcb, start=True, stop=True)
                t3 = work.tile([P, CH], f32, tag="t3")
                nc.vector.tensor_mul(t3, at, gt_c[:, qh, :])
                nc.vector.tensor_mul(ag[:, qh, :], t3, bc)

                # drain interleaved projection / o_proj pieces
                target = ntot * (qh + 1) // QH
                while popped < target:
                    filler.popleft()()
                    popped += 1

        # ---- main schedule ----
        load_xc(0)
        qro_c = qrop.tile([P, QH, CH], bf16, tag="qro")
        gt_c = gtp.tile([P, QH, CH], bf16, tag="gt")
        for piece in proj_pieces(0, qro_c, gt_c):
            piece()
        prev = None
        for c in range(NCH):
            filler = deque()
            qro_n = gt_n = None
            if c + 1 < NCH:
                load_xc(c + 1)
                qro_n = qrop.tile([P, QH, CH], bf16, tag="qro")
                gt_n = gtp.tile([P, QH, CH], bf16, tag="gt")
                filler.extend(proj_pieces(c + 1, qro_n, gt_n))
            if prev is not None:
                filler.extend(oproj_pieces(*prev))
            ag = agp.tile([P, QH, CH], bf16, tag="ag")
            attention(c, qro_c, gt_c, ag, filler)
            while filler:
                filler.popleft()()
            prev = (c, ag)
            qro_c, gt_c = qro_n, gt_n
        for piece in oproj_pieces(*prev):
            piece()

    nc.finalize()
    return nc


_PROGRAMS = {}


def _get_program(S=S_FULL):
    if S not in _PROGRAMS:
        _PROGRAMS[S] = build_program(S)
    return _PROGRAMS[S]


def _host_tables(position_ids_b, S):
    pos = np.asarray(position_ids_b, dtype=np.float32)
    inv = 1.0 / (ROPE_THETA ** (np.arange(0, P, 2, dtype=np.float32) / P))
    ang = np.concatenate([inv, inv]).astype(np.float32)[:, None] * pos[None, :]
    cosT = np.cos(ang).astype(np.float32)
    sgn = np.where(np.arange(P) < 64, -1.0, 1.0).astype(np.float32)
    sinT = (np.sin(ang) * sgn[:, None]).astype(np.float32)
    return cosT, sinT


def _causal_masks():
    o = np.arange(CH // P)[:, None, None]
    r = np.arange(P)[None, :, None]
    j = np.arange(CH)[None, None, :]
    return ((P * o + r) <= j).astype(BF16)


def make_in_maps(x, position_ids, Wq, Wk, Wv, Wo, Wg, bg, S=S_FULL):
    x = np.asarray(x, dtype=np.float32)
    msk = _causal_masks()
    maps = []
    xT_b = [np.ascontiguousarray(x[b, :S].T).astype(BF16) for b in range(B)]
    tabs = [_host_tables(np.asarray(position_ids)[b, :S], S) for b in range(B)]
    Wq = np.asarray(Wq, np.float32)
    Wk = np.asarray(Wk, np.float32)
    Wv = np.asarray(Wv, np.float32)
    Wo = np.asarray(Wo, np.float32)
    Wg = np.asarray(Wg, np.float32)
    bg = np.asarray(bg, np.float32)
    for core in range(8):
        b, g = core // 4, core % 4
        cosT, sinT = tabs[b]
        maps.append({
            "xT": xT_b[b],
            "wq": np.ascontiguousarray(Wq[:, g * DQ:(g + 1) * DQ]).astype(BF16),
            "wk": np.ascontiguousarray(Wk[:, g * P:(g + 1) * P]).astype(BF16),
            "wv": np.ascontiguousarray(VS * Wv[:, g * P:(g + 1) * P]).astype(BF16),
            "wg": np.ascontiguousarray(Wg[:, g * DQ:(g + 1) * DQ]).astype(BF16),
            "wo": np.ascontiguousarray(Wo[g * DQ:(g + 1) * DQ, :]).astype(BF16),
            "bg": np.ascontiguousarray(bg[g * DQ:(g + 1) * DQ]),
            "cosT": cosT,
            "sinT": sinT,
            "msk": msk,
        })
    return maps


def run(inputs, S=S_FULL, trace=False, **kw):
    nc = _get_program(S)
    maps = make_in_maps(S=S, **inputs)
    res = run_bass_kernel_spmd(nc, maps, core_ids=list(range(8)), trace=trace, **kw)
    out = np.zeros((B, S, HIDDEN), np.float32)
    for core in range(8):
        out[core // 4] += np.asarray(res.results[core]["out"], np.float32)
    return out, res


def kernel(x, position_ids, Wq, Wk, Wv, Wo, Wg, bg):
    out, _ = run(dict(x=x, position_ids=position_ids, Wq=Wq, Wk=Wk, Wv=Wv,
                      Wo=Wo, Wg=Wg, bg=bg))
    return out
